# revision 6
# baseline (speedup 1.0000x reference)
"""Bass/Trainium2 kernel for nn_AdaptiveDownsampling (FPS + gather).

Data-parallel over batch B=8: one point cloud per NeuronCore.
Per core: farthest-point-sampling 4096 of 8192 points (sequential argmax
iterations on DVE/GPSIMD/PE), with feature/point gathers via indirect DMA
overlapped with the FPS loop.

kernel(points, features) takes full inputs [8,8192,3]/[8,8192,256] and
returns (dp [8,4096,3], df [8,4096,256]) exactly matching the reference.

Only HW-verified instructions are used (see probe.py): plain DVE ALU ops,
max8/max_index, scalar_tensor_tensor, gpsimd partition_all_reduce (attn
ucode lib), plain PE matmuls (no stride-0 lhsT), indirect DMA with
bounds_check.
"""

import numpy as np

N = 8192
M_SEL = 4096
P = 128
FPP = N // P  # 64 points per partition; flat index i = 64*p + f
C_FEAT = 256
BIG = float(2 ** 20)


def build_nc(n_sel=M_SEL, gather=True):
    import concourse.bass as bass
    import concourse.mybir as mybir
    import concourse.tile as tile
    from concourse import bacc
    from concourse.bass_isa import ReduceOp

    dt = mybir.dt
    Alu = mybir.AluOpType
    Ax = mybir.AxisListType

    nc = bacc.Bacc(None, target_bir_lowering=False)

    points = nc.dram_tensor("points", [N, 3], dt.float32, kind="ExternalInput")
    features = nc.dram_tensor("features", [N, C_FEAT], dt.float32, kind="ExternalInput")
    # iotafp[p, f] = 2^20 - (64*p + f)  (host-supplied constant)
    iotafp_d = nc.dram_tensor("iotafp", [P, FPP], dt.float32, kind="ExternalInput")
    dp = nc.dram_tensor("dp", [n_sel, 3], dt.float32, kind="ExternalOutput")
    df = nc.dram_tensor("df", [n_sel, C_FEAT], dt.float32, kind="ExternalOutput")
    idxo = nc.dram_tensor("idxo", [1, M_SEL], dt.float32, kind="ExternalOutput")

    with tile.TileContext(nc) as tc:
        with (
            tc.tile_pool(name="persist", bufs=1) as pp,
            tc.tile_pool(name="scratch", bufs=2) as sp,
            tc.tile_pool(name="ps", bufs=2, space="PSUM") as ps,
            tc.tile_pool(name="gath", bufs=3) as gp,
        ):
            # ---- persistent state ----
            P3 = pp.tile([P, FPP, 3], dt.float32)
            nc.sync.dma_start(P3[:], points[:].rearrange("(p f) c -> p f c", p=P))
            iotaFP = pp.tile([P, FPP], dt.float32)
            nc.sync.dma_start(iotaFP[:], iotafp_d[:])
            mind = pp.tile([P, FPP], dt.float32)
            nc.vector.memset(mind[:], 1e10)
            idxf = pp.tile([1, M_SEL], dt.float32)
            nc.vector.memset(idxf[:], 0.0)  # also sets idx[0] = 0
            id1 = pp.tile([1, 1], dt.float32)
            nc.vector.memset(id1[:], 1.0)
            ones_row = pp.tile([1, P], dt.float32)
            nc.vector.memset(ones_row[:], 1.0)
            ohp0 = pp.tile([P, 1], dt.float32)
            nc.vector.memset(ohp0[:], 0.0)
            nc.vector.memset(ohp0[0:1, 0:1], 1.0)

            def extract_bcast(oh, rhs):
                """q1 = one-hot row-select(rhs); Qb = q1 broadcast to all parts."""
                q1 = ps.tile([1, 3], dt.float32, tag="q1")
                nc.tensor.matmul(q1[:], lhsT=oh[:], rhs=rhs, start=True, stop=True)
                q1s = sp.tile([1, 3], dt.float32, tag="q1s")
                nc.vector.tensor_copy(q1s[:], q1[:])
                Qb = ps.tile([P, 3], dt.float32, tag="qb")
                nc.tensor.matmul(Qb[:], lhsT=ones_row[:], rhs=q1s[:],
                                 start=True, stop=True)
                return Qb

            # ---- iteration 0: q = pts[0] ----
            Qb_prev = extract_bcast(ohp0, P3[:, 0, :])

            def gather_block(j):
                idxT = ps.tile([P, 1], dt.float32, tag="idxT")
                nc.tensor.transpose(
                    idxT[:], in_=idxf[0:1, P * j : P * (j + 1)], identity=id1[:]
                )
                idxi = gp.tile([P, 1], dt.int32, tag="idxi")
                nc.vector.tensor_copy(idxi[:], idxT[:])
                ftile = gp.tile([P, C_FEAT], dt.float32, tag="ftile")
                nc.gpsimd.indirect_dma_start(
                    out=ftile[:], out_offset=None, in_=features[:],
                    in_offset=bass.IndirectOffsetOnAxis(ap=idxi[:, 0:1], axis=0),
                    bounds_check=N - 1, oob_is_err=False,
                )
                nc.sync.dma_start(df[P * j : P * (j + 1), :], ftile[:])
                ptile = gp.tile([P, 3], dt.float32, tag="ptile")
                nc.gpsimd.indirect_dma_start(
                    out=ptile[:], out_offset=None, in_=points[:],
                    in_offset=bass.IndirectOffsetOnAxis(ap=idxi[:, 0:1], axis=0),
                    bounds_check=N - 1, oob_is_err=False,
                )
                nc.sync.dma_start(dp[P * j : P * (j + 1), :], ptile[:])

            # ---- main FPS loop ----
            for k in range(1, n_sel):
                D3 = sp.tile([P, FPP, 3], dt.float32, tag="D3")
                S3 = sp.tile([P, FPP, 3], dt.float32, tag="S3")
                dsq = sp.tile([P, FPP], dt.float32, tag="dsq")
                m8 = sp.tile([P, 8], dt.float32, tag="m8")
                f8u = sp.tile([P, 8], dt.uint32, tag="f8u")
                f8f = sp.tile([P, 1], dt.float32, tag="f8f")
                W8 = sp.tile([P, 1], dt.float32, tag="W8")
                Mb = sp.tile([P, 1], dt.float32, tag="Mb")
                msk = sp.tile([P, 1], dt.float32, tag="msk")
                G = sp.tile([P, 1], dt.float32, tag="G")
                ohp = sp.tile([P, 1], dt.float32, tag="ohp")
                ohA = sp.tile([P, FPP], dt.float32, tag="ohA")
                T3 = sp.tile([P, FPP, 3], dt.float32, tag="T3")
                candQ = sp.tile([P, 3], dt.float32, tag="candQ")

                # distance to q_{k-1}, running min
                nc.vector.tensor_tensor(
                    out=D3[:], in0=P3[:],
                    in1=Qb_prev[:].unsqueeze(1).to_broadcast([P, FPP, 3]),
                    op=Alu.subtract,
                )
                nc.vector.tensor_tensor(out=S3[:], in0=D3[:], in1=D3[:], op=Alu.mult)
                nc.vector.tensor_reduce(out=dsq[:], in_=S3[:], axis=Ax.X, op=Alu.add)
                nc.vector.tensor_tensor(out=mind[:], in0=mind[:], in1=dsq[:], op=Alu.min)

                # per-partition argmax (first occurrence), W8 = 2^20 - (64p + f*)
                nc.vector.max(out=m8[:], in_=mind[:])
                nc.vector.max_index(out=f8u[:], in_max=m8[:], in_values=mind[:])
                nc.vector.tensor_copy(f8f[:], f8u[:, 0:1])
                nc.vector.tensor_scalar(
                    out=W8[:], in0=f8f[:], scalar1=-1.0, scalar2=iotaFP[:, 0:1],
                    op0=Alu.mult, op1=Alu.add,
                )

                # cross-partition argmax with first-index tie-break
                nc.gpsimd.partition_all_reduce(Mb[:], m8[:, 0:1], P, ReduceOp.max)
                nc.vector.scalar_tensor_tensor(
                    out=msk[:], in0=m8[:, 0:1], scalar=Mb[:, 0:1], in1=W8[:],
                    op0=Alu.is_equal, op1=Alu.mult,
                )
                nc.gpsimd.partition_all_reduce(G[:], msk[:], P, ReduceOp.max)
                nc.vector.tensor_scalar(
                    out=ohp[:], in0=W8[:], scalar1=G[:, 0:1], scalar2=None,
                    op0=Alu.is_equal,
                )

                # selected point's coords
                nc.vector.tensor_scalar(
                    out=ohA[:], in0=iotaFP[:], scalar1=W8[:, 0:1], scalar2=None,
                    op0=Alu.is_equal,
                )
                nc.vector.tensor_tensor(
                    out=T3[:], in0=P3[:],
                    in1=ohA[:].unsqueeze(2).to_broadcast([P, FPP, 3]),
                    op=Alu.mult,
                )
                nc.vector.tensor_reduce(
                    out=candQ[:], in_=T3[:].transpose([0, 2, 1]), axis=Ax.X, op=Alu.add
                )
                Qb_prev = extract_bcast(ohp, candQ[:])

                # append selected index: idx = 2^20 - G
                nc.vector.tensor_scalar(
                    out=idxf[0:1, k : k + 1], in0=G[0:1, 0:1],
                    scalar1=-1.0, scalar2=BIG, op0=Alu.mult, op1=Alu.add,
                )

                if gather and (k + 1) % P == 0:
                    gather_block((k + 1) // P - 1)

            nc.sync.dma_start(idxo[:], idxf[:])

    return nc


def _host_inputs(pts_b):
    """Per-core inputs given that core's points [N, 3]."""
    pf = np.arange(N, dtype=np.float64).reshape(P, FPP)
    iotafp = (BIG - pf).astype(np.float32)
    return {"points": np.ascontiguousarray(pts_b), "iotafp": iotafp}


_NC_CACHE = {}


def _get_nc(n_sel=M_SEL, gather=True):
    key = (n_sel, gather)
    if key not in _NC_CACHE:
        nc = build_nc(n_sel, gather)
        nc.compile()
        _NC_CACHE[key] = nc
    return _NC_CACHE[key]


def kernel(points, features):
    from concourse.bass_utils import run_bass_kernel_spmd

    points = np.asarray(points, dtype=np.float32)
    features = np.asarray(features, dtype=np.float32)
    B = points.shape[0]
    assert points.shape == (B, N, 3) and features.shape == (B, N, C_FEAT)

    nc = _get_nc()
    in_maps = []
    for b in range(B):
        m = _host_inputs(points[b])
        m["features"] = np.ascontiguousarray(features[b])
        in_maps.append(m)

    res = run_bass_kernel_spmd(nc, in_maps, core_ids=list(range(B)))
    dp = np.stack([res.results[b]["dp"] for b in range(B)])
    df = np.stack([res.results[b]["df"] for b in range(B)])
    return dp, df


# revision 10
# speedup vs baseline: 1.0174x; 1.0174x over previous
"""Bass/Trainium2 kernel for nn_AdaptiveDownsampling (FPS + gather).

Data-parallel over batch B=8: one point cloud per NeuronCore.
Per core: farthest-point-sampling 4096 of 8192 points (sequential argmax
iterations on DVE/GPSIMD/PE), with feature/point gathers via indirect DMA
overlapped with the FPS loop.

kernel(points, features) takes full inputs [8,8192,3]/[8,8192,256] and
returns (dp [8,4096,3], df [8,4096,256]) exactly matching the reference.

Only HW-verified instructions are used (see probe.py): plain DVE ALU ops,
max8/max_index, scalar_tensor_tensor, gpsimd partition_all_reduce (attn
ucode lib), plain PE matmuls (no stride-0 lhsT), indirect DMA with
bounds_check.
"""

import numpy as np

N = 8192
M_SEL = 4096
P = 128
FPP = N // P  # 64 points per partition; flat index i = 64*p + f
C_FEAT = 256
BIG = float(2 ** 20)


def build_nc(n_sel=M_SEL, gather=True):
    import concourse.bass as bass
    import concourse.mybir as mybir
    import concourse.tile as tile
    from concourse import bacc
    from concourse.bass_isa import ReduceOp

    dt = mybir.dt
    Alu = mybir.AluOpType
    Ax = mybir.AxisListType
    ActF = mybir.ActivationFunctionType

    nc = bacc.Bacc(None, target_bir_lowering=False)

    points = nc.dram_tensor("points", [N, 3], dt.float32, kind="ExternalInput")
    features = nc.dram_tensor("features", [N, C_FEAT], dt.float32, kind="ExternalInput")
    # iotafp[p, f] = 2^20 - (64*p + f)  (host-supplied constant)
    iotafp_d = nc.dram_tensor("iotafp", [P, FPP], dt.float32, kind="ExternalInput")
    dp = nc.dram_tensor("dp", [n_sel, 3], dt.float32, kind="ExternalOutput")
    df = nc.dram_tensor("df", [n_sel, C_FEAT], dt.float32, kind="ExternalOutput")
    idxo = nc.dram_tensor("idxo", [1, M_SEL], dt.float32, kind="ExternalOutput")

    with tile.TileContext(nc) as tc:
        with (
            tc.tile_pool(name="persist", bufs=1) as pp,
            tc.tile_pool(name="scratch", bufs=2) as sp,
            tc.tile_pool(name="ps", bufs=2, space="PSUM") as ps,
            tc.tile_pool(name="gath", bufs=3) as gp,
        ):
            # ---- persistent state ----
            P3 = pp.tile([P, FPP, 3], dt.float32)
            nc.sync.dma_start(P3[:], points[:].rearrange("(p f) c -> p f c", p=P))
            iotaFP = pp.tile([P, FPP], dt.float32)
            nc.sync.dma_start(iotaFP[:], iotafp_d[:])
            mind = pp.tile([P, FPP], dt.float32)
            nc.vector.memset(mind[:], 1e10)
            idxf = pp.tile([1, M_SEL], dt.float32)
            nc.vector.memset(idxf[:], 0.0)  # also sets idx[0] = 0
            id1 = pp.tile([1, 1], dt.float32)
            nc.vector.memset(id1[:], 1.0)
            ones128 = pp.tile([P, P], dt.float32)
            nc.vector.memset(ones128[:], 1.0)
            ohp0 = pp.tile([P, 1], dt.float32)
            nc.vector.memset(ohp0[:], 0.0)
            nc.vector.memset(ohp0[0:1, 0:1], 1.0)

            def sum_bcast(rhs):
                """Qb[m, c] = sum_p rhs[p, c] for all m (ones matmul).
                With rhs one-hot-masked rows this is extract+broadcast."""
                Qb = ps.tile([P, 3], dt.float32, tag="qb")
                nc.tensor.matmul(Qb[:], lhsT=ones128[:], rhs=rhs,
                                 start=True, stop=True)
                return Qb

            # ---- iteration 0: q = pts[0]: mask P3[:,0,:] rows to partition 0 ----
            c0 = sp.tile([P, 3], dt.float32, tag="candQ")
            nc.vector.tensor_tensor(
                out=c0[:], in0=P3[:, 0, :],
                in1=ohp0[:].to_broadcast([P, 3]), op=Alu.mult,
            )
            Qb_prev = sum_bcast(c0[:])

            def gather_block(j):
                idxT = ps.tile([P, 1], dt.float32, tag="idxT")
                nc.tensor.transpose(
                    idxT[:], in_=idxf[0:1, P * j : P * (j + 1)], identity=id1[:]
                )
                idxi = gp.tile([P, 1], dt.int32, tag="idxi")
                nc.vector.tensor_copy(idxi[:], idxT[:])
                ftile = gp.tile([P, C_FEAT], dt.float32, tag="ftile")
                nc.gpsimd.indirect_dma_start(
                    out=ftile[:], out_offset=None, in_=features[:],
                    in_offset=bass.IndirectOffsetOnAxis(ap=idxi[:, 0:1], axis=0),
                    bounds_check=N - 1, oob_is_err=False,
                )
                nc.sync.dma_start(df[P * j : P * (j + 1), :], ftile[:])
                ptile = gp.tile([P, 3], dt.float32, tag="ptile")
                nc.gpsimd.indirect_dma_start(
                    out=ptile[:], out_offset=None, in_=points[:],
                    in_offset=bass.IndirectOffsetOnAxis(ap=idxi[:, 0:1], axis=0),
                    bounds_check=N - 1, oob_is_err=False,
                )
                nc.sync.dma_start(dp[P * j : P * (j + 1), :], ptile[:])

            # ---- main FPS loop ----
            for k in range(1, n_sel):
                D3 = sp.tile([P, FPP, 3], dt.float32, tag="D3")
                S3 = sp.tile([P, FPP, 3], dt.float32, tag="S3")
                dsq = sp.tile([P, FPP], dt.float32, tag="dsq")
                m8 = sp.tile([P, 8], dt.float32, tag="m8")
                f8u = sp.tile([P, 8], dt.uint32, tag="f8u")
                f8f = sp.tile([P, 1], dt.float32, tag="f8f")
                W8 = sp.tile([P, 1], dt.float32, tag="W8")
                Mb = sp.tile([P, 1], dt.float32, tag="Mb")
                msk = sp.tile([P, 1], dt.float32, tag="msk")
                G = sp.tile([P, 1], dt.float32, tag="G")
                ohA = sp.tile([P, FPP], dt.float32, tag="ohA")
                T3 = sp.tile([P, FPP, 3], dt.float32, tag="T3")
                candQ = sp.tile([P, 3], dt.float32, tag="candQ")

                # distance to q_{k-1}, running min
                nc.vector.tensor_tensor(
                    out=D3[:], in0=P3[:],
                    in1=Qb_prev[:].unsqueeze(1).to_broadcast([P, FPP, 3]),
                    op=Alu.subtract,
                )
                nc.vector.tensor_tensor(out=S3[:], in0=D3[:], in1=D3[:], op=Alu.mult)
                nc.vector.tensor_reduce(out=dsq[:], in_=S3[:], axis=Ax.X, op=Alu.add)
                nc.vector.tensor_tensor(out=mind[:], in0=mind[:], in1=dsq[:], op=Alu.min)

                # per-partition argmax (first occurrence), W8 = 2^20 - (64p + f*)
                nc.vector.max(out=m8[:], in_=mind[:])
                nc.vector.max_index(out=f8u[:], in_max=m8[:], in_values=mind[:])
                nc.vector.tensor_copy(f8f[:], f8u[:, 0:1])
                nc.vector.tensor_scalar(
                    out=W8[:], in0=f8f[:], scalar1=-1.0, scalar2=iotaFP[:, 0:1],
                    op0=Alu.mult, op1=Alu.add,
                )

                # cross-partition argmax with first-index tie-break
                nc.gpsimd.partition_all_reduce(Mb[:], m8[:, 0:1], P, ReduceOp.max)
                nc.vector.scalar_tensor_tensor(
                    out=msk[:], in0=m8[:, 0:1], scalar=Mb[:, 0:1], in1=W8[:],
                    op0=Alu.is_equal, op1=Alu.mult,
                )
                nc.gpsimd.partition_all_reduce(G[:], msk[:], P, ReduceOp.max)

                # global one-hot over all (p, f) directly from G, then coords
                nc.vector.tensor_scalar(
                    out=ohA[:], in0=iotaFP[:], scalar1=G[:, 0:1], scalar2=None,
                    op0=Alu.is_equal,
                )
                nc.vector.tensor_tensor(
                    out=T3[:], in0=P3[:],
                    in1=ohA[:].unsqueeze(2).to_broadcast([P, FPP, 3]),
                    op=Alu.mult,
                )
                nc.vector.tensor_reduce(
                    out=candQ[:], in_=T3[:].transpose([0, 2, 1]), axis=Ax.X, op=Alu.add
                )
                Qb_prev = sum_bcast(candQ[:])

                # append selected index: idx = 2^20 - G (on idle Scalar engine)
                nc.scalar.activation(
                    out=idxf[0:1, k : k + 1], in_=G[0:1, 0:1],
                    func=ActF.Copy, bias=BIG, scale=-1.0,
                )

                if gather and (k + 1) % P == 0:
                    gather_block((k + 1) // P - 1)

            nc.sync.dma_start(idxo[:], idxf[:])

    return nc


def _host_inputs(pts_b):
    """Per-core inputs given that core's points [N, 3]."""
    pf = np.arange(N, dtype=np.float64).reshape(P, FPP)
    iotafp = (BIG - pf).astype(np.float32)
    return {"points": np.ascontiguousarray(pts_b), "iotafp": iotafp}


_NC_CACHE = {}


def _get_nc(n_sel=M_SEL, gather=True):
    key = (n_sel, gather)
    if key not in _NC_CACHE:
        nc = build_nc(n_sel, gather)
        nc.compile()
        _NC_CACHE[key] = nc
    return _NC_CACHE[key]


def kernel(points, features):
    from concourse.bass_utils import run_bass_kernel_spmd

    points = np.asarray(points, dtype=np.float32)
    features = np.asarray(features, dtype=np.float32)
    B = points.shape[0]
    assert points.shape == (B, N, 3) and features.shape == (B, N, C_FEAT)

    nc = _get_nc()
    in_maps = []
    for b in range(B):
        m = _host_inputs(points[b])
        m["features"] = np.ascontiguousarray(features[b])
        in_maps.append(m)

    res = run_bass_kernel_spmd(nc, in_maps, core_ids=list(range(B)))
    dp = np.stack([res.results[b]["dp"] for b in range(B)])
    df = np.stack([res.results[b]["df"] for b in range(B)])
    return dp, df


# revision 12
# speedup vs baseline: 1.0948x; 1.0760x over previous
"""Bass/Trainium2 kernel for nn_AdaptiveDownsampling (FPS + gather).

Data-parallel over batch B=8: one point cloud per NeuronCore.
Per core: farthest-point-sampling 4096 of 8192 points (sequential argmax
iterations on DVE/GPSIMD/PE), with feature/point gathers via indirect DMA
overlapped with the FPS loop.

kernel(points, features) takes full inputs [8,8192,3]/[8,8192,256] and
returns (dp [8,4096,3], df [8,4096,256]) exactly matching the reference.

Only HW-verified instructions are used (see probe.py): plain DVE ALU ops,
max8/max_index, scalar_tensor_tensor, gpsimd partition_all_reduce (attn
ucode lib), plain PE matmuls (no stride-0 lhsT), indirect DMA with
bounds_check.
"""

import numpy as np

N = 8192
M_SEL = 4096
P = 128
FPP = N // P  # 64 points per partition; flat index i = 64*p + f
C_FEAT = 256
BIG = float(2 ** 20)


def build_nc(n_sel=M_SEL, gather=True):
    import concourse.bass as bass
    import concourse.mybir as mybir
    import concourse.tile as tile
    from concourse import bacc
    from concourse.bass_isa import ReduceOp

    dt = mybir.dt
    Alu = mybir.AluOpType
    Ax = mybir.AxisListType
    ActF = mybir.ActivationFunctionType

    nc = bacc.Bacc(None, target_bir_lowering=False)

    points = nc.dram_tensor("points", [N, 3], dt.float32, kind="ExternalInput")
    features = nc.dram_tensor("features", [N, C_FEAT], dt.float32, kind="ExternalInput")
    # iotafp[p, f] = 2^20 - (64*p + f)  (host-supplied constant)
    iotafp_d = nc.dram_tensor("iotafp", [P, FPP], dt.float32, kind="ExternalInput")
    dp = nc.dram_tensor("dp", [n_sel, 3], dt.float32, kind="ExternalOutput")
    df = nc.dram_tensor("df", [n_sel, C_FEAT], dt.float32, kind="ExternalOutput")
    idxo = nc.dram_tensor("idxo", [1, M_SEL], dt.float32, kind="ExternalOutput")

    with tile.TileContext(nc) as tc:
        with (
            tc.tile_pool(name="persist", bufs=1) as pp,
            tc.tile_pool(name="scratch", bufs=2) as sp,
            tc.tile_pool(name="ps", bufs=2, space="PSUM") as ps,
            tc.tile_pool(name="gath", bufs=3) as gp,
        ):
            # ---- persistent state ----
            P3 = pp.tile([P, FPP, 3], dt.float32)
            nc.sync.dma_start(P3[:], points[:].rearrange("(p f) c -> p f c", p=P))
            iotaFP = pp.tile([P, FPP], dt.float32)
            nc.sync.dma_start(iotaFP[:], iotafp_d[:])
            mind = pp.tile([P, FPP], dt.float32)
            nc.vector.memset(mind[:], 1e10)
            idxf = pp.tile([1, M_SEL], dt.float32)
            nc.vector.memset(idxf[:], 0.0)  # also sets idx[0] = 0
            id1 = pp.tile([1, 1], dt.float32)
            nc.vector.memset(id1[:], 1.0)
            ohp0 = pp.tile([P, 1], dt.float32)
            nc.vector.memset(ohp0[:], 0.0)
            nc.vector.memset(ohp0[0:1, 0:1], 1.0)

            def sum_bcast(masked):
                """Qb[m, c] = sum_p masked[p, c] for all m via gpsimd
                all-reduce-add. With one-hot-masked rows: extract+broadcast."""
                Qb = sp.tile([P, 3], dt.float32, tag="qb")
                nc.gpsimd.partition_all_reduce(Qb[:], masked, P, ReduceOp.add)
                return Qb

            # ---- iteration 0: q = pts[0]: mask P3[:,0,:] rows to partition 0 ----
            c0 = sp.tile([P, 3], dt.float32, tag="cmask")
            nc.vector.tensor_tensor(
                out=c0[:], in0=P3[:, 0, :],
                in1=ohp0[:].to_broadcast([P, 3]), op=Alu.mult,
            )
            Qb_prev = sum_bcast(c0[:])

            def gather_block(j):
                idxT = ps.tile([P, 1], dt.float32, tag="idxT")
                nc.tensor.transpose(
                    idxT[:], in_=idxf[0:1, P * j : P * (j + 1)], identity=id1[:]
                )
                idxi = gp.tile([P, 1], dt.int32, tag="idxi")
                nc.vector.tensor_copy(idxi[:], idxT[:])
                ftile = gp.tile([P, C_FEAT], dt.float32, tag="ftile")
                nc.gpsimd.indirect_dma_start(
                    out=ftile[:], out_offset=None, in_=features[:],
                    in_offset=bass.IndirectOffsetOnAxis(ap=idxi[:, 0:1], axis=0),
                    bounds_check=N - 1, oob_is_err=False,
                )
                nc.sync.dma_start(df[P * j : P * (j + 1), :], ftile[:])
                ptile = gp.tile([P, 3], dt.float32, tag="ptile")
                nc.gpsimd.indirect_dma_start(
                    out=ptile[:], out_offset=None, in_=points[:],
                    in_offset=bass.IndirectOffsetOnAxis(ap=idxi[:, 0:1], axis=0),
                    bounds_check=N - 1, oob_is_err=False,
                )
                nc.sync.dma_start(dp[P * j : P * (j + 1), :], ptile[:])

            # ---- main FPS loop ----
            for k in range(1, n_sel):
                D3 = sp.tile([P, FPP, 3], dt.float32, tag="D3")
                S3 = sp.tile([P, FPP, 3], dt.float32, tag="S3")
                dsq = sp.tile([P, FPP], dt.float32, tag="dsq")
                m8 = sp.tile([P, 8], dt.float32, tag="m8")
                f8u = sp.tile([P, 8], dt.uint32, tag="f8u")
                f8f = sp.tile([P, 1], dt.float32, tag="f8f")
                W8 = sp.tile([P, 1], dt.float32, tag="W8")
                Mb = sp.tile([P, 1], dt.float32, tag="Mb")
                msk = sp.tile([P, 1], dt.float32, tag="msk")
                G = sp.tile([P, 1], dt.float32, tag="G")
                ohA = sp.tile([P, FPP], dt.float32, tag="ohA")
                T3 = sp.tile([P, FPP, 3], dt.float32, tag="T3")
                candQ = sp.tile([P, 3], dt.float32, tag="candQ")

                # distance to q_{k-1}, running min
                nc.vector.tensor_tensor(
                    out=D3[:], in0=P3[:],
                    in1=Qb_prev[:].unsqueeze(1).to_broadcast([P, FPP, 3]),
                    op=Alu.subtract,
                )
                nc.vector.tensor_tensor(out=S3[:], in0=D3[:], in1=D3[:], op=Alu.mult)
                nc.vector.tensor_reduce(out=dsq[:], in_=S3[:], axis=Ax.X, op=Alu.add)
                nc.vector.tensor_tensor(out=mind[:], in0=mind[:], in1=dsq[:], op=Alu.min)

                # per-partition argmax (first occurrence), W8 = 2^20 - (64p + f*)
                nc.vector.max(out=m8[:], in_=mind[:])
                nc.vector.max_index(out=f8u[:], in_max=m8[:], in_values=mind[:])
                nc.vector.tensor_copy(f8f[:], f8u[:, 0:1])
                nc.vector.tensor_scalar(
                    out=W8[:], in0=f8f[:], scalar1=-1.0, scalar2=iotaFP[:, 0:1],
                    op0=Alu.mult, op1=Alu.add,
                )

                # cross-partition argmax with first-index tie-break
                nc.gpsimd.partition_all_reduce(Mb[:], m8[:, 0:1], P, ReduceOp.max)
                nc.vector.scalar_tensor_tensor(
                    out=msk[:], in0=m8[:, 0:1], scalar=Mb[:, 0:1], in1=W8[:],
                    op0=Alu.is_equal, op1=Alu.mult,
                )
                nc.gpsimd.partition_all_reduce(G[:], msk[:], P, ReduceOp.max)

                # per-partition candidate coords (runs in the all-reduce shadow)
                nc.vector.tensor_scalar(
                    out=ohA[:], in0=iotaFP[:], scalar1=W8[:, 0:1], scalar2=None,
                    op0=Alu.is_equal,
                )
                nc.vector.tensor_tensor(
                    out=T3[:], in0=P3[:],
                    in1=ohA[:].unsqueeze(2).to_broadcast([P, FPP, 3]),
                    op=Alu.mult,
                )
                nc.vector.tensor_reduce(
                    out=candQ[:], in_=T3[:].transpose([0, 2, 1]), axis=Ax.X, op=Alu.add
                )
                # select winner partition's coords: (W8 == G) * candQ
                cmask = sp.tile([P, 3], dt.float32, tag="cmask")
                nc.vector.scalar_tensor_tensor(
                    out=cmask[:], in0=W8[:].to_broadcast([P, 3]), scalar=G[:, 0:1],
                    in1=candQ[:], op0=Alu.is_equal, op1=Alu.mult,
                )
                Qb_prev = sum_bcast(cmask[:])

                # append selected index: idx = 2^20 - G (on idle Scalar engine)
                nc.scalar.activation(
                    out=idxf[0:1, k : k + 1], in_=G[0:1, 0:1],
                    func=ActF.Copy, bias=BIG, scale=-1.0,
                )

                if gather and (k + 1) % P == 0:
                    gather_block((k + 1) // P - 1)

            nc.sync.dma_start(idxo[:], idxf[:])

    return nc


def _host_inputs(pts_b):
    """Per-core inputs given that core's points [N, 3]."""
    pf = np.arange(N, dtype=np.float64).reshape(P, FPP)
    iotafp = (BIG - pf).astype(np.float32)
    return {"points": np.ascontiguousarray(pts_b), "iotafp": iotafp}


_NC_CACHE = {}


def _get_nc(n_sel=M_SEL, gather=True):
    key = (n_sel, gather)
    if key not in _NC_CACHE:
        nc = build_nc(n_sel, gather)
        nc.compile()
        _NC_CACHE[key] = nc
    return _NC_CACHE[key]


def kernel(points, features):
    from concourse.bass_utils import run_bass_kernel_spmd

    points = np.asarray(points, dtype=np.float32)
    features = np.asarray(features, dtype=np.float32)
    B = points.shape[0]
    assert points.shape == (B, N, 3) and features.shape == (B, N, C_FEAT)

    nc = _get_nc()
    in_maps = []
    for b in range(B):
        m = _host_inputs(points[b])
        m["features"] = np.ascontiguousarray(features[b])
        in_maps.append(m)

    res = run_bass_kernel_spmd(nc, in_maps, core_ids=list(range(B)))
    dp = np.stack([res.results[b]["dp"] for b in range(B)])
    df = np.stack([res.results[b]["df"] for b in range(B)])
    return dp, df


# revision 13
# speedup vs baseline: 1.1496x; 1.0501x over previous
"""Bass/Trainium2 kernel for nn_AdaptiveDownsampling (FPS + gather).

Data-parallel over batch B=8: one point cloud per NeuronCore.
Per core: farthest-point-sampling 4096 of 8192 points (sequential argmax
iterations on DVE/GPSIMD/PE), with feature/point gathers via indirect DMA
overlapped with the FPS loop.

kernel(points, features) takes full inputs [8,8192,3]/[8,8192,256] and
returns (dp [8,4096,3], df [8,4096,256]) exactly matching the reference.

Only HW-verified instructions are used (see probe.py): plain DVE ALU ops,
max8/max_index, scalar_tensor_tensor, gpsimd partition_all_reduce (attn
ucode lib), plain PE matmuls (no stride-0 lhsT), indirect DMA with
bounds_check.
"""

import numpy as np

N = 8192
M_SEL = 4096
P = 128
FPP = N // P  # 64 points per partition; flat index i = 64*p + f
C_FEAT = 256
BIG = float(2 ** 20)


def build_nc(n_sel=M_SEL, gather=True):
    import concourse.bass as bass
    import concourse.mybir as mybir
    import concourse.tile as tile
    from concourse import bacc
    from concourse.bass_isa import ReduceOp

    dt = mybir.dt
    Alu = mybir.AluOpType
    Ax = mybir.AxisListType
    ActF = mybir.ActivationFunctionType

    nc = bacc.Bacc(None, target_bir_lowering=False)

    points = nc.dram_tensor("points", [N, 3], dt.float32, kind="ExternalInput")
    features = nc.dram_tensor("features", [N, C_FEAT], dt.float32, kind="ExternalInput")
    # iotafp[p, f] = 2^20 - (64*p + f)  (host-supplied constant)
    iotafp_d = nc.dram_tensor("iotafp", [P, FPP], dt.float32, kind="ExternalInput")
    dp = nc.dram_tensor("dp", [n_sel, 3], dt.float32, kind="ExternalOutput")
    df = nc.dram_tensor("df", [n_sel, C_FEAT], dt.float32, kind="ExternalOutput")
    idxo = nc.dram_tensor("idxo", [1, M_SEL], dt.float32, kind="ExternalOutput")

    with tile.TileContext(nc) as tc:
        with (
            tc.tile_pool(name="persist", bufs=1) as pp,
            tc.tile_pool(name="scratch", bufs=2) as sp,
            tc.tile_pool(name="ps", bufs=2, space="PSUM") as ps,
            tc.tile_pool(name="gath", bufs=3) as gp,
        ):
            # ---- persistent state ----
            P3 = pp.tile([P, FPP, 3], dt.float32)
            nc.sync.dma_start(P3[:], points[:].rearrange("(p f) c -> p f c", p=P))
            iotaFP = pp.tile([P, FPP], dt.float32)
            nc.sync.dma_start(iotaFP[:], iotafp_d[:])
            mind = pp.tile([P, FPP], dt.float32)
            nc.vector.memset(mind[:], 1e10)
            idxf = pp.tile([1, M_SEL], dt.float32)
            nc.vector.memset(idxf[:], 0.0)  # also sets idx[0] = 0
            id1 = pp.tile([1, 1], dt.float32)
            nc.vector.memset(id1[:], 1.0)
            ohp0 = pp.tile([P, 1], dt.float32)
            nc.vector.memset(ohp0[:], 0.0)
            nc.vector.memset(ohp0[0:1, 0:1], 1.0)

            def sum_bcast(masked):
                """Qb[m, c] = sum_p masked[p, c] for all m via gpsimd
                all-reduce-add. With one-hot-masked rows: extract+broadcast."""
                Qb = sp.tile([P, 3], dt.float32, tag="qb")
                nc.gpsimd.partition_all_reduce(Qb[:], masked, P, ReduceOp.add)
                return Qb

            # ---- iteration 0: q = pts[0]: mask P3[:,0,:] rows to partition 0 ----
            c0 = sp.tile([P, 3], dt.float32, tag="cmask")
            nc.vector.tensor_tensor(
                out=c0[:], in0=P3[:, 0, :],
                in1=ohp0[:].to_broadcast([P, 3]), op=Alu.mult,
            )
            Qb_prev = sum_bcast(c0[:])

            def gather_block(j):
                idxT = ps.tile([P, 1], dt.float32, tag="idxT")
                nc.tensor.transpose(
                    idxT[:], in_=idxf[0:1, P * j : P * (j + 1)], identity=id1[:]
                )
                idxi = gp.tile([P, 1], dt.int32, tag="idxi")
                nc.vector.tensor_copy(idxi[:], idxT[:])
                ftile = gp.tile([P, C_FEAT], dt.float32, tag="ftile")
                nc.gpsimd.indirect_dma_start(
                    out=ftile[:], out_offset=None, in_=features[:],
                    in_offset=bass.IndirectOffsetOnAxis(ap=idxi[:, 0:1], axis=0),
                    bounds_check=N - 1, oob_is_err=False,
                )
                nc.sync.dma_start(df[P * j : P * (j + 1), :], ftile[:])
                ptile = gp.tile([P, 3], dt.float32, tag="ptile")
                nc.gpsimd.indirect_dma_start(
                    out=ptile[:], out_offset=None, in_=points[:],
                    in_offset=bass.IndirectOffsetOnAxis(ap=idxi[:, 0:1], axis=0),
                    bounds_check=N - 1, oob_is_err=False,
                )
                nc.sync.dma_start(dp[P * j : P * (j + 1), :], ptile[:])

            # ---- main FPS loop ----
            for k in range(1, n_sel):
                D3 = sp.tile([P, FPP, 3], dt.float32, tag="D3")
                S3 = sp.tile([P, FPP, 3], dt.float32, tag="S3")
                dsq = sp.tile([P, FPP], dt.float32, tag="dsq")
                m8 = sp.tile([P, 8], dt.float32, tag="m8")
                f8u = sp.tile([P, 8], dt.uint32, tag="f8u")
                f8f = sp.tile([P, 1], dt.float32, tag="f8f")
                W8 = sp.tile([P, 1], dt.float32, tag="W8")
                Mb = sp.tile([P, 1], dt.float32, tag="Mb")
                msk = sp.tile([P, 1], dt.float32, tag="msk")
                G = sp.tile([P, 1], dt.float32, tag="G")
                ohA = sp.tile([P, FPP], dt.float32, tag="ohA")
                T3 = sp.tile([P, FPP, 3], dt.float32, tag="T3")
                candQ = sp.tile([P, 3], dt.float32, tag="candQ")

                # distance to q_{k-1}, running min
                nc.vector.tensor_tensor(
                    out=D3[:], in0=P3[:],
                    in1=Qb_prev[:].unsqueeze(1).to_broadcast([P, FPP, 3]),
                    op=Alu.subtract,
                )
                nc.vector.tensor_tensor(out=S3[:], in0=D3[:], in1=D3[:], op=Alu.mult)
                nc.vector.tensor_reduce(out=dsq[:], in_=S3[:], axis=Ax.X, op=Alu.add)
                nc.vector.tensor_tensor(out=mind[:], in0=mind[:], in1=dsq[:], op=Alu.min)

                # per-partition argmax (first occurrence), W8 = 2^20 - (64p + f*)
                nc.vector.max(out=m8[:], in_=mind[:])
                nc.vector.max_index(out=f8u[:], in_max=m8[:], in_values=mind[:])
                nc.vector.tensor_copy(f8f[:], f8u[:, 0:1])
                nc.vector.tensor_scalar(
                    out=W8[:], in0=f8f[:], scalar1=-1.0, scalar2=iotaFP[:, 0:1],
                    op0=Alu.mult, op1=Alu.add,
                )

                # cross-partition argmax with first-index tie-break
                nc.gpsimd.partition_all_reduce(Mb[:], m8[:, 0:1], P, ReduceOp.max)
                nc.vector.tensor_scalar(
                    out=ohA[:], in0=iotaFP[:], scalar1=W8[:, 0:1], scalar2=None,
                    op0=Alu.is_equal,
                )
                nc.vector.scalar_tensor_tensor(
                    out=msk[:], in0=m8[:, 0:1], scalar=Mb[:, 0:1], in1=W8[:],
                    op0=Alu.is_equal, op1=Alu.mult,
                )
                nc.gpsimd.partition_all_reduce(G[:], msk[:], P, ReduceOp.max)

                nc.vector.tensor_tensor(
                    out=T3[:], in0=P3[:],
                    in1=ohA[:].unsqueeze(2).to_broadcast([P, FPP, 3]),
                    op=Alu.mult,
                )
                nc.vector.tensor_reduce(
                    out=candQ[:], in_=T3[:].transpose([0, 2, 1]), axis=Ax.X, op=Alu.add
                )
                # select winner partition's coords: (W8 == G) * candQ
                cmask = sp.tile([P, 3], dt.float32, tag="cmask")
                nc.vector.scalar_tensor_tensor(
                    out=cmask[:], in0=W8[:].to_broadcast([P, 3]), scalar=G[:, 0:1],
                    in1=candQ[:], op0=Alu.is_equal, op1=Alu.mult,
                )
                Qb_prev = sum_bcast(cmask[:])

                # append selected index: idx = 2^20 - G (on idle Scalar engine)
                nc.scalar.activation(
                    out=idxf[0:1, k : k + 1], in_=G[0:1, 0:1],
                    func=ActF.Copy, bias=BIG, scale=-1.0,
                )

                if gather and (k + 1) % P == 0:
                    gather_block((k + 1) // P - 1)

            nc.sync.dma_start(idxo[:], idxf[:])

    return nc


def _host_inputs(pts_b):
    """Per-core inputs given that core's points [N, 3]."""
    pf = np.arange(N, dtype=np.float64).reshape(P, FPP)
    iotafp = (BIG - pf).astype(np.float32)
    return {"points": np.ascontiguousarray(pts_b), "iotafp": iotafp}


_NC_CACHE = {}


def _get_nc(n_sel=M_SEL, gather=True):
    key = (n_sel, gather)
    if key not in _NC_CACHE:
        nc = build_nc(n_sel, gather)
        nc.compile()
        _NC_CACHE[key] = nc
    return _NC_CACHE[key]


def kernel(points, features):
    from concourse.bass_utils import run_bass_kernel_spmd

    points = np.asarray(points, dtype=np.float32)
    features = np.asarray(features, dtype=np.float32)
    B = points.shape[0]
    assert points.shape == (B, N, 3) and features.shape == (B, N, C_FEAT)

    nc = _get_nc()
    in_maps = []
    for b in range(B):
        m = _host_inputs(points[b])
        m["features"] = np.ascontiguousarray(features[b])
        in_maps.append(m)

    res = run_bass_kernel_spmd(nc, in_maps, core_ids=list(range(B)))
    dp = np.stack([res.results[b]["dp"] for b in range(B)])
    df = np.stack([res.results[b]["df"] for b in range(B)])
    return dp, df
